# revision 7
# baseline (speedup 1.0000x reference)
"""Trainium2 Bass kernel for MemoryBankNet loss (scatter_memory).

Computes, for inputs/backbone_inputs [256,512], targets [256], memory_features
[100000,512]:
    ce   = cross_entropy(l2norm(inputs) @ mem.T / 0.05, targets)
    dist = (0.007/0.3) * ||l2norm(backbone_inputs) - mem[targets[j//4]]||_F
    out  = ce + dist                                    (f32 scalar)

Distribution: classes (mem rows) are sharded 12500/core across 8 NeuronCores
(tensor parallel over the class axis).  Each core computes its partial softmax
denominator with a fixed log-shift; the tiny [256] partials are combined on
host (the "all-reduce" of the softmax normalizer).  The B target rows are
routed on host (gather mem[targets]) and the dot-products/distill partials are
computed on device.

Device layout per core:
  memT [512, 12500] f32 : transposed mem shard, streamed in 25 strips of
                          [512, 500] (1.02 MB per DMA)
  matmul: stationary = raw transposed inputs inpT [128d x 128b] (float32r),
          moving = memT strip chunk [128d x 500c] (float32r),
          psum [128b, 500c] accumulates over the 4 d-chunks.
  ACT: exp(psum * (20/||inp_b||) - 104) with accum_out -> per-row partial sums.
  l2 normalization is folded into the per-partition activation scale.
"""

import numpy as np

import concourse.bass as bass
import concourse.tile as tile
from concourse import bacc, mybir
from concourse.bass_utils import run_bass_kernel_spmd

F32 = mybir.dt.float32
F32R = mybir.dt.float32r
AF = mybir.ActivationFunctionType
AX = mybir.AxisListType

N_CORES = 8
B, D, C = 256, 512, 100000
CS = C // N_CORES            # 12500 classes per core
KD = D // 128                # 4 contraction chunks
CT = 500                     # classes per strip
NSTRIP = CS // CT            # 25
GROUP = 2                    # strips sharing one weight-load round

TEMP = 0.05
ISCALE = 1.0 / TEMP          # 20.0
SHIFT = 104.0                # fixed log-shift: max |logit| ~ 96 whp
DISTILL_SCALE = 0.007 / 0.3
EPS = 1e-12

_PROGRAM = None


def _build_program():
    nc = bacc.Bacc("TRN2", target_bir_lowering=False, debug=False,
                   num_devices=N_CORES)
    memT = nc.dram_tensor("memT", [D, CS], F32R, kind="ExternalInput").ap()
    inpT = nc.dram_tensor("inpT", [D, B], F32R, kind="ExternalInput").ap()
    inp = nc.dram_tensor("inp", [B, D], F32, kind="ExternalInput").ap()
    bbi = nc.dram_tensor("bbi", [B, D], F32, kind="ExternalInput").ap()
    g1 = nc.dram_tensor("g1", [B, D], F32, kind="ExternalInput").ap()
    g2 = nc.dram_tensor("g2", [B, D], F32, kind="ExternalInput").ap()
    # packed per-core result, col 0-1: sumexp halves, 2-3: target logit halves,
    # 4-5: distill ssq partial halves
    out = nc.dram_tensor("out", [128, 6], F32, kind="ExternalOutput").ap()

    with tile.TileContext(nc) as tc:
        _body(tc, nc, memT, inpT, inp, bbi, g1, g2, out)

    nc.compile()
    return nc


def _body(tc, nc, memT, inpT, inp, bbi, g1, g2, out):
    with (
        tc.tile_pool(name="const", bufs=1) as cpool,
        tc.tile_pool(name="small", bufs=6) as spool,
        tc.tile_pool(name="mstrip", bufs=4) as mpool,
        tc.tile_pool(name="exps", bufs=4) as epool,
        tc.tile_pool(name="psum", bufs=8, space="PSUM") as ppool,
    ):
        # ---- persistent tiles -------------------------------------------
        itb = cpool.tile([128, KD * B], F32R, tag="itb", name="itb")          # inpT chunks
        ibuf = [cpool.tile([128, D], F32, tag=f"ibuf{h}", name=f"ibuf{h}") for h in range(2)]
        bbuf = [cpool.tile([128, D], F32, tag=f"bbuf{h}", name=f"bbuf{h}") for h in range(2)]
        g1b = [cpool.tile([128, D], F32, tag=f"g1b{h}", name=f"g1b{h}") for h in range(2)]
        g2b = [cpool.tile([128, D], F32, tag=f"g2b{h}", name=f"g2b{h}") for h in range(2)]
        res = cpool.tile([128, 6], F32, tag="res", name="res")
        scl = cpool.tile([128, 2], F32, tag="scl", name="scl")               # 20/||inp||
        bscl = cpool.tile([128, 2], F32, tag="bscl", name="bscl")             # 1/||bb||
        nbias = cpool.tile([128, 1], F32, tag="nbias", name="nbias")         # -SHIFT
        nc.vector.memset(nbias[:], -SHIFT)

        # ---- input DMAs -------------------------------------------------
        # big streaming DMAs ride the SP HWDGE ring; small ones the ACT ring
        nc.scalar.dma_start(itb[:].rearrange("p (k b) -> p k b", k=KD),
                            inpT.rearrange("(k p) b -> p k b", p=128))
        for h in range(2):
            rows = slice(h * 128, (h + 1) * 128)
            nc.scalar.dma_start(ibuf[h][:], inp[rows, :])
            nc.scalar.dma_start(bbuf[h][:], bbi[rows, :])
            nc.scalar.dma_start(g1b[h][:], g1[rows, :])
            nc.scalar.dma_start(g2b[h][:], g2[rows, :])

        # ---- row norms -> activation scales -----------------------------
        nc.vector.memset(res[:], 0.0)
        for h in range(2):
            sq = spool.tile([128, D], F32, tag="sq", name="sq")
            ss = spool.tile([128, 1], F32, tag="ss", name="ss")
            nc.scalar.activation(sq[:], ibuf[h][:], AF.Square, accum_out=ss[:])
            nrm = spool.tile([128, 1], F32, tag="nrm", name="nrm")
            nc.scalar.sqrt(nrm[:], ss[:])
            nrm2 = spool.tile([128, 1], F32, tag="nrm2", name="nrm2")
            nc.vector.tensor_scalar_max(nrm2[:], nrm[:], EPS)
            rcp = spool.tile([128, 1], F32, tag="rcp", name="rcp")
            nc.vector.reciprocal(rcp[:], nrm2[:])
            nc.vector.tensor_scalar_mul(scl[:, h:h + 1], rcp[:], ISCALE)

            sqb = spool.tile([128, D], F32, tag="sqb", name="sqb")
            ssb = spool.tile([128, 1], F32, tag="ssb", name="ssb")
            nc.scalar.activation(sqb[:], bbuf[h][:], AF.Square, accum_out=ssb[:])
            nrmb = spool.tile([128, 1], F32, tag="nrmb", name="nrmb")
            nc.scalar.sqrt(nrmb[:], ssb[:])
            nrmb2 = spool.tile([128, 1], F32, tag="nrmb2", name="nrmb2")
            nc.vector.tensor_scalar_max(nrmb2[:], nrmb[:], EPS)
            nc.vector.reciprocal(bscl[:, h:h + 1], nrmb2[:])

        # ---- main loop: stream mem shard, matmul, exp-accumulate --------
        memT_v = memT.rearrange("(k p) c -> p k c", p=128)
        n_groups = (NSTRIP + GROUP - 1) // GROUP
        for g in range(n_groups):
            strips = [s for s in range(g * GROUP, min((g + 1) * GROUP, NSTRIP))]
            mts = []
            for s in strips:
                mt = mpool.tile([128, KD * CT], F32R, tag="mt", name="mt")
                nc.sync.dma_start(
                    mt[:].rearrange("p (k c) -> p k c", k=KD),
                    memT_v[:, :, s * CT:(s + 1) * CT])
                mts.append(mt)
            pss = [[ppool.tile([128, CT], F32, tag="ps", name="ps") for _ in range(2)]
                   for _ in strips]
            for k in range(KD):
                for h in range(2):
                    w = itb[:, k * B + h * 128: k * B + (h + 1) * 128]
                    for si in range(len(strips)):
                        nc.tensor.matmul(
                            pss[si][h][:],
                            w,
                            mts[si][:, k * CT:(k + 1) * CT],
                            start=(k == 0), stop=(k == KD - 1))
            for si in range(len(strips)):
                for h in range(2):
                    ex = epool.tile([128, CT], F32, tag="ex", name="ex")
                    pacc = spool.tile([128, 1], F32, tag="pacc", name="pacc")
                    nc.scalar.activation(ex[:], pss[si][h][:], AF.Exp,
                                         bias=nbias[:], scale=scl[:, h:h + 1],
                                         accum_out=pacc[:])
                    nc.vector.tensor_add(res[:, h:h + 1], res[:, h:h + 1],
                                         pacc[:])

        # ---- target logits: sum(inp * g1) * (20/||inp||) ----------------
        for h in range(2):
            prod = spool.tile([128, D], F32, tag="prod", name="prod")
            tlr = spool.tile([128, 1], F32, tag="tlr", name="tlr")
            nc.vector.tensor_mul(prod[:], ibuf[h][:], g1b[h][:])
            nc.vector.reduce_sum(tlr[:], prod[:], axis=AX.X)
            nc.vector.tensor_mul(res[:, 2 + h:3 + h], tlr[:], scl[:, h:h + 1])

        # ---- distill partials: sum((bb/||bb|| - g2)^2) per row ----------
        for h in range(2):
            bbn = spool.tile([128, D], F32, tag="bbn", name="bbn")
            nc.vector.tensor_scalar_mul(bbn[:], bbuf[h][:], bscl[:, h:h + 1])
            diff = spool.tile([128, D], F32, tag="diff", name="diff")
            nc.vector.tensor_sub(diff[:], bbn[:], g2b[h][:])
            sqd = spool.tile([128, D], F32, tag="sqd", name="sqd")
            nc.scalar.activation(sqd[:], diff[:], AF.Square,
                                 accum_out=res[:, 4 + h:5 + h])

        nc.scalar.dma_start(out, res[:, 0:6])


def _get_program():
    global _PROGRAM
    if _PROGRAM is None:
        _PROGRAM = _build_program()
    return _PROGRAM


def kernel(backbone_inputs, inputs, targets, memory_features, **_unused):
    x = np.ascontiguousarray(inputs, dtype=np.float32)
    bb = np.ascontiguousarray(backbone_inputs, dtype=np.float32)
    mem = np.ascontiguousarray(memory_features, dtype=np.float32)
    tgt = np.asarray(targets).astype(np.int64)

    # host-side routing of the B target rows
    g1 = np.ascontiguousarray(mem[tgt])                                # [256,512]
    g2 = np.ascontiguousarray(mem[tgt[np.arange(B) // 4]])             # [256,512]
    xT = np.ascontiguousarray(x.T)                                     # [512,256]

    nc = _get_program()
    in_maps = []
    for c in range(N_CORES):
        shard = np.ascontiguousarray(mem[c * CS:(c + 1) * CS].T)       # [512,12500]
        in_maps.append({
            "memT": shard,
            "inpT": xT,
            "inp": x,
            "bbi": bb,
            "g1": g1,
            "g2": g2,
        })
    results = run_bass_kernel_spmd(nc, in_maps, core_ids=list(range(N_CORES)))

    outs = [r["out"] for r in results.results]                         # [128,6] each
    s_tot = np.zeros(B, dtype=np.float64)
    for o in outs:
        s_tot += np.concatenate([o[:, 0], o[:, 1]]).astype(np.float64)
    o0 = outs[0]
    tl = np.concatenate([o0[:, 2], o0[:, 3]]).astype(np.float64)       # target logits
    ssq = float(np.concatenate([o0[:, 4], o0[:, 5]]).astype(np.float64).sum())

    lse = SHIFT + np.log(s_tot)                                        # logsumexp
    ce = float(np.mean(lse - tl))
    dist = DISTILL_SCALE * float(np.sqrt(ssq))
    return np.float32(ce + dist)
